# revision 3
# baseline (speedup 1.0000x reference)
"""Multi-head attention (B=2, L=2048, H=16, D=64) on 8 TRN2 NeuronCores.

Sharding: core = (batch b, head-group hg); 2 batches x 4 groups of 4 heads.
Per core, for its batch and its 4 heads (2 head-pairs):
    Q^T/K^T = W^T x^T           (d on partitions; pair m: head 2m at rows
                                 0:64, head 2m+1 at rows 64:128)
    V       = x W_v             (j on partitions, + ones column for denom)
    S^T     = K^T.T Q^T         (j on partitions, i free)
    P'      = exp(S^T/8)        (un-normalized softmax numerator)
    O'^T    = [V|1].T P'        (row 64 = softmax denominator)
    O^T     = O'[0:64] * (1/O'[64])
    out^T  += Wo_rows^T O^T     (partial over head-group rows of Wo)
Host sums the 4 partials per batch, transposes, adds bo.

All matmul operands are float16 (fp16 streams at 1 cyc/row on the PE at
2.4 GHz warm vs ~2x slower for f32r; PSUM accumulation stays f32).  The
attention j-loop is ACT(exp)-bound at ~2.3us per j, so everything else
is scheduled around keeping the scalar engine saturated:

  - x^T is DMA'd in [128,1024] chunks, low seq-half first, and Q/K
    projections for the first half run k-chunk-major across 4 PSUM
    tiles so the PE tracks DMA arrival; a few dummy matmuls on Wq warm
    the PE HAM clock-gate before real work lands.
  - unit order is (ih,m) = (0,0),(1,0),(0,1),(1,1): the m=1 projections
    are interleaved one PSUM-tile at a time into the ACT-bound slack of
    units 0-1, and the ih=0 output projection into unit 3's slack.
  - each unit's softmax normalization is deferred into the next unit's
    j-loop (reciprocal chain hides under ~2 j-steps of exp).
  - only unit 3's normalize + the ih=1 output projection remain as the
    serial tail.
"""

import sys

try:
    import concourse.bass as bass  # noqa: F401
except ImportError:  # pragma: no cover - path fallback
    sys.path.insert(0, "/opt/trn_rl_repo")

import numpy as np
import concourse.bass as bass
import concourse.mybir as mybir
import concourse.tile as tile
from concourse import bacc
from concourse.bass_utils import run_bass_kernel_spmd

F32 = mybir.dt.float32
F16 = mybir.dt.float16
AF = mybir.ActivationFunctionType

B = 2
L = 2048          # sequence length
C = 1024          # model dim
H_LOC = 4         # heads per core
D = 64            # head dim
HD = H_LOC * D    # 256 = local head-group width
KT = C // 128     # 8 k-tiles over the model dim
SCALE2 = float(D) ** -0.5  # 1/8, applied once inside exp

_cache = {}


def _build():
    nc = bacc.Bacc("TRN2", target_bir_lowering=False, debug=False, num_devices=8)

    xT = nc.declare_dram_parameter("xT", [C, L], F16, isOutput=False)
    wq = nc.declare_dram_parameter("wq", [C, HD], F16, isOutput=False)
    wk = nc.declare_dram_parameter("wk", [C, HD], F16, isOutput=False)
    wv = nc.declare_dram_parameter("wv", [C, HD], F16, isOutput=False)
    wo = nc.declare_dram_parameter("wo", [HD, C], F16, isOutput=False)
    outT = nc.declare_dram_parameter("outT", [C, L], F32, isOutput=True)

    with tile.TileContext(nc) as tc:
        with tc.tile_pool(name="sb", bufs=1) as sb, \
             tc.tile_pool(name="ps", bufs=2, space="PSUM") as ps, \
             tc.tile_pool(name="po", bufs=2, space="PSUM") as po:

            # ---- input DMA: weights first, then x^T low-half chunk-major ---
            wq_sb = sb.tile([128, KT, HD], F16, tag="wq")
            wk_sb = sb.tile([128, KT, HD], F16, tag="wk")
            wv_sb = sb.tile([128, KT, HD], F16, tag="wv")
            wo_sb = sb.tile([128, 2, C], F16, tag="wo")
            xT_sb = sb.tile([128, KT, L], F16, tag="xT")
            nc.sync.dma_start(wq_sb[:, :, :], wq.rearrange("(k p) c -> p k c", p=128))
            nc.sync.dma_start(wk_sb[:, :, :], wk.rearrange("(k p) c -> p k c", p=128))
            for k in range(KT):
                nc.sync.dma_start(xT_sb[:, k, 0:1024],
                                  xT[k * 128:(k + 1) * 128, 0:1024])
            nc.sync.dma_start(wv_sb[:, :, :], wv.rearrange("(k p) c -> p k c", p=128))
            nc.sync.dma_start(wo_sb[:, :, :], wo.rearrange("(k p) c -> p k c", p=128))
            for k in range(KT):
                nc.sync.dma_start(xT_sb[:, k, 1024:2048],
                                  xT[k * 128:(k + 1) * 128, 1024:2048])

            ones_f = sb.tile([128, 64], F32, tag="ones_f")
            nc.vector.memset(ones_f[:], 1.0)
            warm_junk = sb.tile([128, 8], F32, tag="warm_junk")

            qT_sb = sb.tile([128, 2, L], F16, tag="qT")
            kT_sb = sb.tile([128, 2, L], F16, tag="kT")
            v_sb = sb.tile([128, 16, H_LOC, D + 1], F16, tag="v")
            oT_sb = sb.tile([128, 2, L], F16, tag="oT")

            # ---- PE warm-up: dummy matmuls on wq keep the HAM clock-gate
            # busy while the x^T DMA streams in --------------------------
            for g in range(3):
                pd = ps.tile([128, 1024], F32, tag="s", name="warm")
                for i in range(6):
                    nc.tensor.matmul(
                        pd[:, 0:HD],
                        wq_sb[:, 0, 0:128],
                        wq_sb[:, 0, :],
                        start=(i == 0), stop=(i == 5),
                    )
                nc.vector.tensor_copy(warm_junk[:], pd[:, 0:8])

            def copy_lp(dst, src):
                with nc.allow_low_precision(reason="fp16 matmul input"):
                    nc.vector.tensor_copy(dst, src)

            # ---- Q/K projections m=0, seq-half 0: k-chunk-major across 4
            # PSUM tiles (2 from ps + 2 borrowed from po) so the PE keeps
            # pace with the x^T chunk arrivals -----------------------------
            pk0 = ps.tile([128, 1024], F32, tag="s", name="pk0")
            pk1 = ps.tile([128, 1024], F32, tag="s", name="pk1")
            pq0 = po.tile([128, 1024], F32, tag="o", name="pq0")
            pq1 = po.tile([128, 1024], F32, tag="o", name="pq1")
            for k in range(KT):
                for acc, w_sb, n in ((pk0, wk_sb, 0), (pk1, wk_sb, 1),
                                     (pq0, wq_sb, 0), (pq1, wq_sb, 1)):
                    nc.tensor.matmul(
                        acc[:, 0:512],
                        w_sb[:, k, 0:128],
                        xT_sb[:, k, n * 512:(n + 1) * 512],
                        start=(k == 0), stop=(k == KT - 1),
                    )
            copy_lp(kT_sb[:, 0, 0:512], pk0[:, 0:512])
            copy_lp(kT_sb[:, 0, 512:1024], pk1[:, 0:512])
            copy_lp(qT_sb[:, 0, 0:512], pq0[:, 0:512])
            copy_lp(qT_sb[:, 0, 512:1024], pq1[:, 0:512])

            # V with ones column: v_sb[p, j_tile, h, 0:64]=V, [..., 64]=1
            copy_lp(
                v_sb[:, :, :, D:D + 1],
                ones_f.rearrange("p (a b c) -> p a b c", a=16, b=4),
            )

            def emit_v_tile(it):
                p = po.tile([128, 1024], F32, tag="o", name="vp")
                acc = p[:, 0:HD]
                for k in range(KT):
                    nc.tensor.matmul(
                        acc,
                        xT_sb[:, k, it * 128:(it + 1) * 128],
                        wv_sb[:, k, :],
                        start=(k == 0), stop=(k == KT - 1),
                    )
                copy_lp(
                    v_sb[:, it, :, 0:D],
                    acc.rearrange("p (h d) -> p h d", h=H_LOC),
                )

            def emit_proj_tile(w_sb, t_sb, m, n):
                p = ps.tile([128, 1024], F32, tag="s", name="proj")
                acc = p[:, 0:512]
                for k in range(KT):
                    nc.tensor.matmul(
                        acc,
                        w_sb[:, k, m * 128:(m + 1) * 128],
                        xT_sb[:, k, n * 512:(n + 1) * 512],
                        start=(k == 0), stop=(k == KT - 1),
                    )
                copy_lp(t_sb[:, m, n * 512:(n + 1) * 512], acc)

            # V for seq-half 0 (j-tiles 0-7), then K^T m=0 half 1, then the
            # rest of V; q^T m=0 half 1 and everything m=1 interleave into
            # the attention loops below.
            for it in range(8):
                emit_v_tile(it)
            emit_proj_tile(wk_sb, kT_sb, 0, 2)
            emit_proj_tile(wk_sb, kT_sb, 0, 3)
            for it in range(8, 16):
                emit_v_tile(it)

            es_pool = tc.alloc_tile_pool(name="es_pool", bufs=6)
            st_pool = tc.alloc_tile_pool(name="st_pool", bufs=2)
            ost_pool = tc.alloc_tile_pool(name="ost_pool", bufs=6)
            np_pool = tc.alloc_tile_pool(name="np_pool", bufs=3)
            d0_pool = tc.alloc_tile_pool(name="d0_pool", bufs=1)

            pending = []   # deferred normalize: (m, i0, o_cps, d0s)

            def emit_normalize():
                m, i0, o_cps, d0s = pending.pop(0)
                for hl in range(2):
                    rep_sb = st_pool.tile([64, 1024], F32, tag="rep")
                    nc.gpsimd.partition_broadcast(rep_sb[:], d0s[hl][:])
                    with nc.allow_low_precision(reason="fp16 matmul input"):
                        if hl == 0:
                            nc.vector.tensor_mul(
                                oT_sb[0:64, m, i0:i0 + 1024],
                                o_cps[hl][0:64, :], rep_sb[:])
                        else:
                            stage = st_pool.tile([64, 1024], F16, tag="stage")
                            nc.vector.tensor_mul(
                                stage[:], o_cps[hl][0:64, :], rep_sb[:])
                            nc.gpsimd.dma_start(
                                oT_sb[64:128, m, i0:i0 + 1024], stage[:])

            def emit_wo_chunk(ih, ct):
                # [128, 1024] output chunk; kk-outer reuses each Wo k-tile's
                # LDWEIGHTS across both 512-wide matmuls
                i0 = ih * 1024
                acc = ps.tile([128, 1024], F32, tag="s", name="wo_ps")
                for kk in range(2):
                    for n in range(2):
                        nc.tensor.matmul(
                            acc[:, n * 512:(n + 1) * 512],
                            wo_sb[:, kk, ct * 128:(ct + 1) * 128],
                            oT_sb[:, kk, i0 + n * 512:i0 + (n + 1) * 512],
                            start=(kk == 0), stop=(kk == 1),
                        )
                ost = ost_pool.tile([128, 1024], F32, tag="ost", name="ost")
                nc.vector.tensor_copy(ost[:], acc[:])
                nc.sync.dma_start(
                    outT[ct * 128:(ct + 1) * 128, i0:i0 + 1024], ost[:])

            # per-unit interleave schedule: j-step -> list of closures.
            # m=1 projection tiles land before their consumers (unit 2 needs
            # qT m1 n0/n1 + kT m1 n0 at start, kT m1 n(s) by its j=4s; unit 3
            # needs qT m1 n2/n3); wo(ih=0) fills unit 3's slack.
            interleave = [dict() for _ in range(4)]
            interleave[0] = {
                2: [lambda: emit_proj_tile(wq_sb, qT_sb, 0, 2)],
                5: [lambda: emit_proj_tile(wq_sb, qT_sb, 0, 3)],
                8: [lambda: emit_proj_tile(wq_sb, qT_sb, 1, 0)],
                11: [lambda: emit_proj_tile(wq_sb, qT_sb, 1, 1)],
                14: [lambda: emit_proj_tile(wk_sb, kT_sb, 1, 0)],
            }
            interleave[1] = {
                2: [lambda: emit_proj_tile(wk_sb, kT_sb, 1, 1)],
                5: [lambda: emit_proj_tile(wk_sb, kT_sb, 1, 2)],
                8: [emit_normalize,
                    lambda: emit_proj_tile(wk_sb, kT_sb, 1, 3)],
                11: [lambda: emit_proj_tile(wq_sb, qT_sb, 1, 2)],
                14: [lambda: emit_proj_tile(wq_sb, qT_sb, 1, 3)],
            }
            interleave[2] = {
                8: [emit_normalize],
            }
            interleave[3] = {
                2: [emit_normalize],
                3: [lambda: emit_wo_chunk(0, 0)],
                5: [lambda: emit_wo_chunk(0, 1)],
                7: [lambda: emit_wo_chunk(0, 2)],
                9: [lambda: emit_wo_chunk(0, 3)],
                10: [lambda: emit_wo_chunk(0, 4)],
                12: [lambda: emit_wo_chunk(0, 5)],
                13: [lambda: emit_wo_chunk(0, 6)],
                15: [lambda: emit_wo_chunk(0, 7)],
            }

            units = [(0, 0), (1, 0), (0, 1), (1, 1)]  # (ih, m)
            for ui, (ih, m) in enumerate(units):
                i0 = ih * 1024
                o_h = []
                for hl in range(2):
                    of = po.tile([128, 1024], F32, tag="o", name=f"o_ps{hl}")
                    o_h.append(of[0:65, :])
                es = [None, None]
                for j in range(16):
                    s_list = []
                    for hl in range(2):
                        r0 = hl * 64
                        s_ps = ps.tile([128, 1024], F32, tag="s", name=f"s_ps{hl}")
                        for n in range(2):
                            nc.tensor.matmul(
                                s_ps[:, n * 512:(n + 1) * 512],
                                kT_sb[r0:r0 + 64, m, j * 128:(j + 1) * 128],
                                qT_sb[r0:r0 + 64, m,
                                      i0 + n * 512:i0 + (n + 1) * 512],
                                start=True, stop=True,
                            )
                        s_list.append(s_ps)
                    prev_es = es
                    es = []
                    for hl in range(2):
                        e_sb = es_pool.tile([128, 1024], F16, tag="es",
                                            name=f"es{hl}")
                        with nc.allow_low_precision(reason="fp16 matmul input"):
                            nc.scalar.activation(e_sb[:], s_list[hl][:], AF.Exp,
                                                 scale=SCALE2)
                        es.append(e_sb)
                    # AV for step j-1 (software-pipelined one step behind)
                    if j > 0:
                        for hl in range(2):
                            for n in range(2):
                                nc.tensor.matmul(
                                    o_h[hl][:, n * 512:(n + 1) * 512],
                                    v_sb[:, j - 1, 2 * m + hl, :],
                                    prev_es[hl][:, n * 512:(n + 1) * 512],
                                    start=(j == 1), stop=False,
                                )
                    for task in interleave[ui].get(j, ()):
                        task()
                # epilogue AV for j=15
                for hl in range(2):
                    for n in range(2):
                        nc.tensor.matmul(
                            o_h[hl][:, n * 512:(n + 1) * 512],
                            v_sb[:, 15, 2 * m + hl, :],
                            es[hl][:, n * 512:(n + 1) * 512],
                            start=False, stop=True,
                        )
                # pull O' off PSUM, reciprocal in place on the denominator
                # row, defer the DVE-side normalize
                o_cps, d0s = [], []
                for hl in range(2):
                    o_cp = np_pool.tile([65, 1024], F32, tag="o_cp",
                                        name=f"o_cp{hl}")
                    nc.vector.tensor_copy(o_cp[:], o_h[hl][:])
                    # reshape the denominator row across all 128 lanes so the
                    # reciprocal runs at full DVE width, then reshape back
                    dsq = d0_pool.tile([128, 8], F32, tag=f"dsq_{hl}")
                    nc.gpsimd.dma_start(dsq[:], o_cp[64:65, :])
                    nc.vector.reciprocal(dsq[:], dsq[:])
                    d0 = d0_pool.tile([1, 1024], F32, tag=f"d0_{hl}")
                    nc.gpsimd.dma_start(d0[:], dsq[:])
                    o_cps.append(o_cp)
                    d0s.append(d0)
                pending.append((m, i0, o_cps, d0s))

            # tail: unit 3's normalize + the ih=1 output projection
            while pending:
                emit_normalize()
            for ct in range(8):
                emit_wo_chunk(1, ct)

            d0_pool.release()
            np_pool.release()
            ost_pool.release()
            st_pool.release()
            es_pool.release()

    nc.compile()
    return nc


def kernel(x, Wq, Wk, Wv, Wo, bo):
    x = np.asarray(x, dtype=np.float32)
    Wq = np.asarray(Wq, dtype=np.float32)
    Wk = np.asarray(Wk, dtype=np.float32)
    Wv = np.asarray(Wv, dtype=np.float32)
    Wo = np.asarray(Wo, dtype=np.float32)
    bo = np.asarray(bo, dtype=np.float32)

    if "nc" not in _cache:
        _cache["nc"] = _build()
    nc = _cache["nc"]

    xTs = [np.ascontiguousarray(x[b].T).astype(np.float16) for b in range(B)]
    in_maps = []
    for core in range(8):
        b, hg = divmod(core, 4)
        sl = slice(hg * HD, (hg + 1) * HD)
        in_maps.append({
            "xT": xTs[b],
            "wq": np.ascontiguousarray(Wq[:, sl]).astype(np.float16),
            "wk": np.ascontiguousarray(Wk[:, sl]).astype(np.float16),
            "wv": np.ascontiguousarray(Wv[:, sl]).astype(np.float16),
            "wo": np.ascontiguousarray(Wo[sl, :]).astype(np.float16),
        })

    res = run_bass_kernel_spmd(nc, in_maps, core_ids=list(range(8)))
    out = np.empty((B, L, C), dtype=np.float32)
    for b in range(B):
        acc = res.results[4 * b]["outT"]
        for hg in range(1, 4):
            acc = acc + res.results[4 * b + hg]["outT"]
        out[b] = acc.T + bo
    return out
